# revision 3
# baseline (speedup 1.0000x reference)
"""Binarized linear + BatchNorm (eval) on 8 Trainium2 NeuronCores.

Computes: out = BN(sign(x) @ sign(weight).T)  for
  x [8192, 4096] f32, weight [4096, 4096] f32, BN vectors [4096] f32.

Strategy
--------
Sharding: batch 4-way x out_features 2-way (8 cores). Each core gets
  xt = x_shard.T   [4096(IN), 2048(B)]  f32   (transposed layout so the
  wt = w_shard.T   [4096(IN), 2048(O)]  f32    contraction dim lands on
                                               SBUF partitions)
and computes outT = [2048(O), 2048(B)] f32 = BN(sign(W).sign(X)) locally.
No collectives; host concatenates the 8 tiles.

Per-core kernel: sign() is done on the scalar engine (ACT) straight to
fp8e4. Since sign(x) in {-1,+1} is exact in fp8 and the PE accumulates in
fp32 PSUM, the binary matmul in fp8 with DoubleRow perf mode (2 MACs/
cell/cycle, K=256 per matmul) is bit-exact. BatchNorm folds to
out = a*acc + b with a = gamma/sqrt(var+eps), b = beta - mean*a (computed
on-device), applied by the vector engine during PSUM drain.

Dataflow: W is streamed once, binarized, and kept fp8-resident in SBUF
(64KB/partition). X is binarized into a second resident fp8 buffer.
Batch is processed in two waves (1536 + 512 cols): wave 0 overlaps the
one-time W load/binarize with matmuls; wave 1 runs from resident data.
"""

import numpy as np
from contextlib import ExitStack

B_FULL, IN, OUT = 8192, 4096, 4096
NB_CORES = 8
BI, OI = 4, 2            # batch x out_features core grid
BS = B_FULL // BI        # 2048 batch per core
OS = OUT // OI           # 2048 out_features per core
KT = IN // 128           # 32 k-tiles of 128
NS = KT // 2             # 16 k256 supertiles (DoubleRow)
OT = OS // 128           # 16 out tiles of 128
NBT = BS // 512          # 4 batch tiles of 512
BN_EPS = 1e-05

# batch tiles per wave: wave0 overlaps the W stream, wave1 is pure PE
WAVES = [list(range(3)), [3]]

_CACHE = {}


def _build_program():
    import concourse.tile as tile
    from concourse import mybir, bacc

    F8 = mybir.dt.float8e4
    F32 = mybir.dt.float32
    Sign = mybir.ActivationFunctionType.Sign
    Sqrt = mybir.ActivationFunctionType.Sqrt
    DR = mybir.MatmulPerfMode.DoubleRow

    nc = bacc.Bacc("TRN2", target_bir_lowering=False, debug=False,
                   num_devices=NB_CORES)
    xt = nc.declare_dram_parameter("xt", [IN, BS], F32, isOutput=False)
    wt = nc.declare_dram_parameter("wt", [IN, OS], F32, isOutput=False)
    g = nc.declare_dram_parameter("g", [OS], F32, isOutput=False)
    bt = nc.declare_dram_parameter("bt", [OS], F32, isOutput=False)
    mu = nc.declare_dram_parameter("mu", [OS], F32, isOutput=False)
    vr = nc.declare_dram_parameter("vr", [OS], F32, isOutput=False)
    o = nc.declare_dram_parameter("o", [OS, BS], F32, isOutput=True)

    with tile.TileContext(nc) as tc:
        with ExitStack() as ctx:
            cons = ctx.enter_context(tc.tile_pool(name="cons", bufs=1))
            xsp = ctx.enter_context(tc.tile_pool(name="xs", bufs=3))
            wsp = ctx.enter_context(tc.tile_pool(name="ws", bufs=3))
            obp = ctx.enter_context(tc.tile_pool(name="ob", bufs=6))
            psp = ctx.enter_context(tc.tile_pool(name="ps", bufs=8, space="PSUM"))

            # ---- BN constants: a = g/sqrt(var+eps), b = beta - mean*a,
            # laid out [128, OT] so column ot is the per-partition vector
            # for out-tile ot.
            gs = cons.tile([128, OT], F32)
            bs_ = cons.tile([128, OT], F32)
            ms = cons.tile([128, OT], F32)
            vs = cons.tile([128, OT], F32)
            for src, dst in ((g, gs), (bt, bs_), (mu, ms), (vr, vs)):
                nc.sync.dma_start(dst[:], src.rearrange("(t p) -> p t", p=128))
            a_sb = cons.tile([128, OT], F32)
            b_sb = cons.tile([128, OT], F32)
            std = cons.tile([128, OT], F32)
            eps = cons.tile([128, 1], F32)
            nc.vector.memset(eps[:], BN_EPS)
            nc.scalar.activation(std[:], vs[:], Sqrt, bias=eps[:, 0:1])
            nc.vector.reciprocal(std[:], std[:])
            nc.vector.tensor_mul(a_sb[:], gs[:], std[:])
            nc.vector.tensor_mul(b_sb[:], ms[:], a_sb[:])
            nc.vector.tensor_sub(b_sb[:], bs_[:], b_sb[:])

            # ---- resident fp8 operands
            xb = cons.tile([128, KT, BS], F8)    # 64KB/partition
            wb = cons.tile([128, KT, OS], F8)    # 64KB/partition

            w0 = WAVES[0]
            w0n = len(w0) * 512

            # X wave-0 binarize (kt-ordered so PE can start early)
            for kt in range(KT):
                xs = xsp.tile([128, w0n], F32, tag="x0")
                nc.sync.dma_start(xs[:], xt[kt * 128:(kt + 1) * 128, 0:w0n])
                nc.scalar.activation(xb[:, kt, 0:w0n], xs[:], Sign)

            def load_w(ot):
                # stream one [IN, 128] column block of W, sign to fp8
                for h in range(2):
                    ws = wsp.tile([128, 16, 128], F32, tag="w0")
                    src = wt[:, ot * 128:(ot + 1) * 128]
                    src = src.rearrange("(kt p) q -> p kt q", p=128)
                    nc.sync.dma_start(ws[:], src[:, h * 16:(h + 1) * 16, :])
                    nc.scalar.activation(
                        wb[:, h * 16:(h + 1) * 16, ot * 128:(ot + 1) * 128],
                        ws[:], Sign)

            def load_x_wave1(kt_list):
                lo = w0n
                for kt in kt_list:
                    xs = xsp.tile([128, BS - lo], F32, tag="x1")
                    nc.sync.dma_start(xs[:], xt[kt * 128:(kt + 1) * 128, lo:BS])
                    nc.scalar.activation(xb[:, kt, lo:BS], xs[:], Sign)

            def do_block(ot, nbs):
                ps_tiles = []
                for nb in nbs:
                    acc = psp.tile([128, 512], F32, tag="acc", name=f"acc_{ot}_{nb}")
                    ps_tiles.append(acc)
                for s in range(NS):
                    lhsT = wb[:, 2 * s:2 * s + 2, ot * 128:(ot + 1) * 128]
                    for i, nb in enumerate(nbs):
                        rhs = xb[:, 2 * s:2 * s + 2, nb * 512:(nb + 1) * 512]
                        nc.tensor.matmul(ps_tiles[i][:], lhsT, rhs,
                                         start=(s == 0), stop=(s == NS - 1),
                                         perf_mode=DR)
                for i, nb in enumerate(nbs):
                    ob = obp.tile([128, 512], F32, tag="ob")
                    nc.vector.tensor_scalar(
                        ob[:], ps_tiles[i][:],
                        a_sb[:, ot:ot + 1], b_sb[:, ot:ot + 1],
                        mybir.AluOpType.mult, mybir.AluOpType.add)
                    nc.sync.dma_start(
                        o[ot * 128:(ot + 1) * 128, nb * 512:(nb + 1) * 512],
                        ob[:])

            # wave 0: stream W in, 3 batch tiles of matmul per out-tile
            for ot in range(OT):
                load_w(ot)
                do_block(ot, WAVES[0])
                # trickle in wave-1 X during the back half of wave 0
                if len(WAVES) > 1 and 8 <= ot < 12:
                    load_x_wave1(range((ot - 8) * 8, (ot - 7) * 8))
            # wave 1: pure PE from resident operands
            if len(WAVES) > 1:
                for ot in range(OT):
                    do_block(ot, WAVES[1])

    nc.compile()
    return nc


def kernel(x, weight, bn_gamma, bn_beta, bn_mean, bn_var):
    from concourse.bass_utils import run_bass_kernel_spmd

    x = np.asarray(x, dtype=np.float32)
    weight = np.asarray(weight, dtype=np.float32)
    bn_gamma = np.asarray(bn_gamma, dtype=np.float32)
    bn_beta = np.asarray(bn_beta, dtype=np.float32)
    bn_mean = np.asarray(bn_mean, dtype=np.float32)
    bn_var = np.asarray(bn_var, dtype=np.float32)

    if "nc" not in _CACHE:
        _CACHE["nc"] = _build_program()
    nc = _CACHE["nc"]

    xt = [np.ascontiguousarray(x[bi * BS:(bi + 1) * BS, :].T) for bi in range(BI)]
    wt = [np.ascontiguousarray(weight[oi * OS:(oi + 1) * OS, :].T)
          for oi in range(OI)]
    in_maps = []
    for c in range(NB_CORES):
        bi, oi = divmod(c, OI)
        sl = slice(oi * OS, (oi + 1) * OS)
        in_maps.append({
            "xt": xt[bi], "wt": wt[oi],
            "g": bn_gamma[sl], "bt": bn_beta[sl],
            "mu": bn_mean[sl], "vr": bn_var[sl],
        })

    res = run_bass_kernel_spmd(nc, in_maps, list(range(NB_CORES)))
    _CACHE["last_results"] = res

    out = np.empty((B_FULL, OUT), dtype=np.float32)
    for c in range(NB_CORES):
        bi, oi = divmod(c, OI)
        out[bi * BS:(bi + 1) * BS, oi * OS:(oi + 1) * OS] = res.results[c]["o"].T
    return out


# revision 6
# speedup vs baseline: 1.1442x; 1.1442x over previous
"""Binarized linear + BatchNorm (eval) on 8 Trainium2 NeuronCores.

Computes: out = BN(sign(x) @ sign(weight).T)  for
  x [8192, 4096] f32, weight [4096, 4096] f32, BN vectors [4096] f32.

Strategy
--------
Sharding: batch 4-way x out_features 2-way (8 cores). Each core gets a
transposed X shard (contraction dim IN on SBUF partitions), a W shard
pre-tiled into [ot, 128, kt, 128] blocks (16KB contiguous per partition
per block -> efficient DMA), and computes outT [2048(O), 2048(B)] f32
locally. No collectives; the host concatenates the 8 tiles.

Per-core: sign() on the scalar engine straight to fp8e4. sign(x) in
{-1,+1} is exact in fp8 and the PE accumulates in fp32 PSUM, so the
binary matmul in fp8 DoubleRow mode (K=256/matmul, 2x bf16 rate) is
bit-exact. BN folds to out = a*acc + b (a = gamma/sqrt(var+eps),
b = beta - mean*a, computed on-device) applied by the vector engine
during PSUM drain.

Dataflow: X is binarized wave-by-wave (4 batch waves of 512 cols) into a
resident fp8 buffer; W streams once through sign into a resident fp8
buffer during wave 0. DMA rings: X on sync HWDGE, W on scalar HWDGE
(independent FIFOs -> W is not stuck behind X), outputs on gpsimd SWDGE.
"""

import numpy as np
from contextlib import ExitStack

B_FULL, IN, OUT = 8192, 4096, 4096
NB_CORES = 8
BI, OI = 4, 2            # batch x out_features core grid
BS = B_FULL // BI        # 2048 batch per core
OS = OUT // OI           # 2048 out_features per core
KT = IN // 128           # 32 k-tiles of 128
NS = KT // 2             # 16 k256 supertiles (DoubleRow)
OT = OS // 128           # 16 out tiles of 128
NBT = BS // 512          # 4 batch tiles of 512
BN_EPS = 1e-05

_CACHE = {}


def _build_program():
    import concourse.tile as tile
    from concourse import mybir, bacc

    F8 = mybir.dt.float8e4
    F32 = mybir.dt.float32
    Sign = mybir.ActivationFunctionType.Sign
    Sqrt = mybir.ActivationFunctionType.Sqrt
    DR = mybir.MatmulPerfMode.DoubleRow

    nc = bacc.Bacc("TRN2", target_bir_lowering=False, debug=False,
                   num_devices=NB_CORES)
    xt = nc.declare_dram_parameter("xt", [IN, BS], F32, isOutput=False)
    w4 = nc.declare_dram_parameter("w4", [OT, 128, KT, 128], F32, isOutput=False)
    g = nc.declare_dram_parameter("g", [OS], F32, isOutput=False)
    bt = nc.declare_dram_parameter("bt", [OS], F32, isOutput=False)
    mu = nc.declare_dram_parameter("mu", [OS], F32, isOutput=False)
    vr = nc.declare_dram_parameter("vr", [OS], F32, isOutput=False)
    o = nc.declare_dram_parameter("o", [OS, BS], F32, isOutput=True)

    with tile.TileContext(nc) as tc:
        with ExitStack() as ctx:
            cons = ctx.enter_context(tc.tile_pool(name="cons", bufs=1))
            xsp = ctx.enter_context(tc.tile_pool(name="xs", bufs=4))
            wsp = ctx.enter_context(tc.tile_pool(name="ws", bufs=4))
            obp = ctx.enter_context(tc.tile_pool(name="ob", bufs=6))
            psp = ctx.enter_context(tc.tile_pool(name="ps", bufs=8, space="PSUM"))

            # ---- BN constants: a = g/sqrt(var+eps), b = beta - mean*a,
            # laid out [128, OT]: column ot = per-partition vector for
            # out-tile ot.
            gs = cons.tile([128, OT], F32)
            bs_ = cons.tile([128, OT], F32)
            ms = cons.tile([128, OT], F32)
            vs = cons.tile([128, OT], F32)
            for src, dst in ((g, gs), (bt, bs_), (mu, ms), (vr, vs)):
                nc.sync.dma_start(dst[:], src.rearrange("(t p) -> p t", p=128))
            a_sb = cons.tile([128, OT], F32)
            b_sb = cons.tile([128, OT], F32)
            std = cons.tile([128, OT], F32)
            eps = cons.tile([128, 1], F32)
            nc.vector.memset(eps[:], BN_EPS)
            nc.scalar.activation(std[:], vs[:], Sqrt, bias=eps[:, 0:1])
            nc.vector.reciprocal(std[:], std[:])
            nc.vector.tensor_mul(a_sb[:], gs[:], std[:])
            nc.vector.tensor_mul(b_sb[:], ms[:], a_sb[:])
            nc.vector.tensor_sub(b_sb[:], bs_[:], b_sb[:])

            # ---- resident fp8 operands
            xb = cons.tile([128, KT, BS], F8)    # 64KB/partition
            wb = cons.tile([128, KT, OS], F8)    # 64KB/partition

            def w_dma(ot):
                # one [128, 16, 128] f32 half-block; 8KB contiguous/partition
                halves = []
                for h in range(2):
                    ws = wsp.tile([128, 16, 128], F32, tag="w0",
                                  name=f"ws_{ot}_{h}")
                    nc.scalar.dma_start(ws[:], w4[ot, :, h * 16:(h + 1) * 16, :])
                    halves.append(ws)
                return halves

            def w_sign(ot, halves):
                for h, ws in enumerate(halves):
                    nc.scalar.activation(
                        wb[:, h * 16:(h + 1) * 16, ot * 128:(ot + 1) * 128],
                        ws[:], Sign)

            def x_load(w, kt):
                xs = xsp.tile([128, 512], F32, tag="x0", name=f"xs_{w}_{kt}")
                nc.sync.dma_start(xs[:], xt[kt * 128:(kt + 1) * 128,
                                            w * 512:(w + 1) * 512])
                nc.scalar.activation(xb[:, kt, w * 512:(w + 1) * 512],
                                     xs[:], Sign)

            def do_block(ot, nb):
                acc = psp.tile([128, 512], F32, tag="acc", name=f"acc_{ot}_{nb}")
                for s in range(NS):
                    nc.tensor.matmul(
                        acc[:],
                        wb[:, 2 * s:2 * s + 2, ot * 128:(ot + 1) * 128],
                        xb[:, 2 * s:2 * s + 2, nb * 512:(nb + 1) * 512],
                        start=(s == 0), stop=(s == NS - 1),
                        perf_mode=DR)
                ob = obp.tile([128, 512], F32, tag="ob", name=f"ob_{ot}_{nb}")
                nc.vector.tensor_scalar(
                    ob[:], acc[:],
                    a_sb[:, ot:ot + 1], b_sb[:, ot:ot + 1],
                    mybir.AluOpType.mult, mybir.AluOpType.add)
                nc.gpsimd.dma_start(
                    o[ot * 128:(ot + 1) * 128, nb * 512:(nb + 1) * 512], ob[:])

            # ring the first W doorbells before the X-sign stream occupies
            # the scalar engine, so W loads in parallel with X wave 0
            w_pending = {0: w_dma(0), 1: w_dma(1)}

            # wave 0: X wave-0 in, W streamed + made resident, matmul nb=0
            for kt in range(KT):
                x_load(0, kt)
            for ot in range(OT):
                w_sign(ot, w_pending.pop(ot))
                if ot + 2 < OT:
                    w_pending[ot + 2] = w_dma(ot + 2)
                do_block(ot, 0)

            # waves 1-3: X wave in, matmul from resident wb
            for w in range(1, NBT):
                for kt in range(KT):
                    x_load(w, kt)
                for ot in range(OT):
                    do_block(ot, w)

    nc.compile()
    return nc


def make_in_maps(x, weight, bn_gamma, bn_beta, bn_mean, bn_var):
    xt = [np.ascontiguousarray(x[bi * BS:(bi + 1) * BS, :].T) for bi in range(BI)]
    # W pre-tiling: w4[ot, p, kt, q] = weight[oi*OS + ot*128 + q, kt*128 + p]
    w4 = []
    for oi in range(OI):
        ws = weight[oi * OS:(oi + 1) * OS, :]          # [OS(O), IN]
        t = ws.reshape(OT, 128, KT, 128)               # [ot, q, kt, p]
        w4.append(np.ascontiguousarray(t.transpose(0, 3, 2, 1)))
    in_maps = []
    for c in range(NB_CORES):
        bi, oi = divmod(c, OI)
        sl = slice(oi * OS, (oi + 1) * OS)
        in_maps.append({
            "xt": xt[bi], "w4": w4[oi],
            "g": bn_gamma[sl], "bt": bn_beta[sl],
            "mu": bn_mean[sl], "vr": bn_var[sl],
        })
    return in_maps


def kernel(x, weight, bn_gamma, bn_beta, bn_mean, bn_var):
    from concourse.bass_utils import run_bass_kernel_spmd

    x = np.asarray(x, dtype=np.float32)
    weight = np.asarray(weight, dtype=np.float32)
    bn_gamma = np.asarray(bn_gamma, dtype=np.float32)
    bn_beta = np.asarray(bn_beta, dtype=np.float32)
    bn_mean = np.asarray(bn_mean, dtype=np.float32)
    bn_var = np.asarray(bn_var, dtype=np.float32)

    if "nc" not in _CACHE:
        _CACHE["nc"] = _build_program()
    nc = _CACHE["nc"]

    in_maps = make_in_maps(x, weight, bn_gamma, bn_beta, bn_mean, bn_var)

    res = run_bass_kernel_spmd(nc, in_maps, list(range(NB_CORES)))
    _CACHE["last_results"] = res

    out = np.empty((B_FULL, OUT), dtype=np.float32)
    for c in range(NB_CORES):
        bi, oi = divmod(c, OI)
        out[bi * BS:(bi + 1) * BS, oi * OS:(oi + 1) * OS] = res.results[c]["o"].T
    return out


# revision 10
# speedup vs baseline: 1.1891x; 1.0392x over previous
"""Binarized linear + BatchNorm (eval) on 8 Trainium2 NeuronCores.

Computes: out = BN(sign(x) @ sign(weight).T)  for
  x [8192, 4096] f32, weight [4096, 4096] f32, BN vectors [4096] f32.

Strategy
--------
Sharding: batch 4-way x out_features 2-way (8 cores). Each core gets a
transposed X shard (contraction dim IN on SBUF partitions), a W shard
pre-tiled into [ot, 128, kt, 128] blocks (16KB contiguous per partition
per block -> efficient DMA), and computes outT [2048(O), 2048(B)] f32
locally. No collectives; the host concatenates the 8 tiles.

Per-core: sign() on the scalar engine straight to fp8e4. sign(x) in
{-1,+1} is exact in fp8 and the PE accumulates in fp32 PSUM, so the
binary matmul in fp8 DoubleRow mode (K=256/matmul, 2x bf16 rate) is
bit-exact. BN folds to out = a*acc + b (a = gamma/sqrt(var+eps),
b = beta - mean*a, computed on-device) applied by the vector engine
during PSUM drain.

Dataflow: X is binarized wave-by-wave (4 batch waves of 512 cols) into a
resident fp8 buffer; W streams once through sign into a resident fp8
buffer during wave 0. DMA rings: X on sync HWDGE, W on scalar HWDGE
(independent FIFOs -> W is not stuck behind X), outputs on gpsimd SWDGE.
"""

import numpy as np
from contextlib import ExitStack

B_FULL, IN, OUT = 8192, 4096, 4096
NB_CORES = 8
BI, OI = 4, 2            # batch x out_features core grid
BS = B_FULL // BI        # 2048 batch per core
OS = OUT // OI           # 2048 out_features per core
KT = IN // 128           # 32 k-tiles of 128
NS = KT // 2             # 16 k256 supertiles (DoubleRow)
OT = OS // 128           # 16 out tiles of 128
NBT = BS // 512          # 4 batch tiles of 512
BN_EPS = 1e-05

_CACHE = {}


def _build_program():
    import concourse.tile as tile
    from concourse import mybir, bacc

    F8 = mybir.dt.float8e4
    F32 = mybir.dt.float32
    Sign = mybir.ActivationFunctionType.Sign
    Sqrt = mybir.ActivationFunctionType.Sqrt
    DR = mybir.MatmulPerfMode.DoubleRow

    nc = bacc.Bacc("TRN2", target_bir_lowering=False, debug=False,
                   num_devices=NB_CORES)
    xt = nc.declare_dram_parameter("xt", [IN, BS], F32, isOutput=False)
    w4 = nc.declare_dram_parameter("w4", [OT, 128, KT, 128], F32, isOutput=False)
    g = nc.declare_dram_parameter("g", [OS], F32, isOutput=False)
    bt = nc.declare_dram_parameter("bt", [OS], F32, isOutput=False)
    mu = nc.declare_dram_parameter("mu", [OS], F32, isOutput=False)
    vr = nc.declare_dram_parameter("vr", [OS], F32, isOutput=False)
    o = nc.declare_dram_parameter("o", [OS, BS], F32, isOutput=True)

    with tile.TileContext(nc) as tc:
        with ExitStack() as ctx:
            cons = ctx.enter_context(tc.tile_pool(name="cons", bufs=1))
            xsp = ctx.enter_context(tc.tile_pool(name="xs", bufs=5))
            wsp = ctx.enter_context(tc.tile_pool(name="ws", bufs=4))
            obp = ctx.enter_context(tc.tile_pool(name="ob", bufs=4))
            psp = ctx.enter_context(tc.tile_pool(name="ps", bufs=8, space="PSUM"))

            # ---- BN constants: a = g/sqrt(var+eps), b = beta - mean*a,
            # laid out [128, OT]: column ot = per-partition vector for
            # out-tile ot.
            gs = cons.tile([128, OT], F32)
            bs_ = cons.tile([128, OT], F32)
            ms = cons.tile([128, OT], F32)
            vs = cons.tile([128, OT], F32)
            for src, dst in ((g, gs), (bt, bs_), (mu, ms), (vr, vs)):
                nc.gpsimd.dma_start(dst[:], src.rearrange("(t p) -> p t", p=128))
            a_sb = cons.tile([128, OT], F32)
            b_sb = cons.tile([128, OT], F32)
            std = cons.tile([128, OT], F32)
            eps = cons.tile([128, 1], F32)
            nc.vector.memset(eps[:], BN_EPS)
            nc.scalar.activation(std[:], vs[:], Sqrt, bias=eps[:, 0:1])
            nc.vector.reciprocal(std[:], std[:])
            nc.vector.tensor_mul(a_sb[:], gs[:], std[:])
            nc.vector.tensor_mul(b_sb[:], ms[:], a_sb[:])
            nc.vector.tensor_sub(b_sb[:], bs_[:], b_sb[:])

            # ---- resident fp8 operands
            xb = cons.tile([128, KT, BS], F8)    # 64KB/partition
            wb = cons.tile([128, KT, OS], F8)    # 64KB/partition

            w_pending = {}

            def w_dma(ot):
                # two [128, 16, 128] f32 half-blocks; 8KB contig/partition
                halves = []
                for h in range(2):
                    ws = wsp.tile([128, 16, 128], F32, tag="w0",
                                  name=f"ws_{ot}_{h}")
                    nc.scalar.dma_start(ws[:], w4[ot, :, h * 16:(h + 1) * 16, :])
                    halves.append(ws)
                w_pending[ot] = halves

            def w_sign(ot):
                for h, ws in enumerate(w_pending.pop(ot)):
                    nc.scalar.activation(
                        wb[:, h * 16:(h + 1) * 16, ot * 128:(ot + 1) * 128],
                        ws[:], Sign)

            def x_load(pair, kt):
                # [128, 1024] f32 chunk covering two 512-wide batch tiles
                xs = xsp.tile([128, 1024], F32, tag="x0", name=f"xs_{pair}_{kt}")
                nc.sync.dma_start(xs[:], xt[kt * 128:(kt + 1) * 128,
                                            pair * 1024:(pair + 1) * 1024])
                nc.scalar.activation(xb[:, kt, pair * 1024:(pair + 1) * 1024],
                                     xs[:], Sign)

            def do_block(ot, nb):
                acc = psp.tile([128, 512], F32, tag="acc", name=f"acc_{ot}_{nb}")
                for s in range(NS):
                    nc.tensor.matmul(
                        acc[:],
                        wb[:, 2 * s:2 * s + 2, ot * 128:(ot + 1) * 128],
                        xb[:, 2 * s:2 * s + 2, nb * 512:(nb + 1) * 512],
                        start=(s == 0), stop=(s == NS - 1),
                        perf_mode=DR)
                ob = obp.tile([128, 512], F32, tag="ob", name=f"ob_{ot}_{nb}")
                nc.vector.tensor_scalar(
                    ob[:], acc[:],
                    a_sb[:, ot:ot + 1], b_sb[:, ot:ot + 1],
                    mybir.AluOpType.mult, mybir.AluOpType.add)
                nc.gpsimd.dma_start(
                    o[ot * 128:(ot + 1) * 128, nb * 512:(nb + 1) * 512], ob[:])

            # W for the first two out-tiles up front (own HWDGE ring), sign
            # decoupled from the DMA so the scalar engine barely stalls
            w_dma(0)
            w_sign(0)
            w_dma(1)
            # X wave-pair 0 (cols 0:1024), kt-ordered so the PE can start
            # as soon as the first supertile and wb[ot0] are signed; W for
            # out-tiles 2..7 trickles through the scalar stream (dma and
            # sign offset by 3 kt steps so the sign never waits)
            w_dma_at = {2: 2}
            w_sign_at = {5: 1}
            for i, ot in enumerate(range(3, 8)):
                w_dma_at[8 + 3 * i] = ot
                w_sign_at[11 + 3 * i] = ot - 1
            w_sign_at[26] = 7
            for kt in range(KT):
                x_load(0, kt)
                if kt in w_dma_at:
                    w_dma(w_dma_at[kt])
                if kt in w_sign_at:
                    w_sign(w_sign_at[kt])

            # wave 0: batch tiles 0,1 per out-tile; W ot 8..15 loads ride
            # along (scalar engine is half idle now)
            for ot in range(OT):
                if ot + 8 < OT:
                    w_dma(ot + 8)
                    w_sign(ot + 8)
                do_block(ot, 0)
                do_block(ot, 1)

            # waves 1-2: batch tiles 2,3 from resident wb
            for kt in range(KT):
                x_load(1, kt)
            for ot in range(OT):
                do_block(ot, 2)
                do_block(ot, 3)

    nc.compile()
    return nc


def make_in_maps(x, weight, bn_gamma, bn_beta, bn_mean, bn_var):
    xt = [np.ascontiguousarray(x[bi * BS:(bi + 1) * BS, :].T) for bi in range(BI)]
    # W pre-tiling: w4[ot, p, kt, q] = weight[oi*OS + ot*128 + q, kt*128 + p]
    w4 = []
    for oi in range(OI):
        ws = weight[oi * OS:(oi + 1) * OS, :]          # [OS(O), IN]
        t = ws.reshape(OT, 128, KT, 128)               # [ot, q, kt, p]
        w4.append(np.ascontiguousarray(t.transpose(0, 3, 2, 1)))
    in_maps = []
    for c in range(NB_CORES):
        bi, oi = divmod(c, OI)
        sl = slice(oi * OS, (oi + 1) * OS)
        in_maps.append({
            "xt": xt[bi], "w4": w4[oi],
            "g": bn_gamma[sl], "bt": bn_beta[sl],
            "mu": bn_mean[sl], "vr": bn_var[sl],
        })
    return in_maps


def kernel(x, weight, bn_gamma, bn_beta, bn_mean, bn_var):
    from concourse.bass_utils import run_bass_kernel_spmd

    x = np.asarray(x, dtype=np.float32)
    weight = np.asarray(weight, dtype=np.float32)
    bn_gamma = np.asarray(bn_gamma, dtype=np.float32)
    bn_beta = np.asarray(bn_beta, dtype=np.float32)
    bn_mean = np.asarray(bn_mean, dtype=np.float32)
    bn_var = np.asarray(bn_var, dtype=np.float32)

    if "nc" not in _CACHE:
        _CACHE["nc"] = _build_program()
    nc = _CACHE["nc"]

    in_maps = make_in_maps(x, weight, bn_gamma, bn_beta, bn_mean, bn_var)

    res = run_bass_kernel_spmd(nc, in_maps, list(range(NB_CORES)))
    _CACHE["last_results"] = res

    out = np.empty((B_FULL, OUT), dtype=np.float32)
    for c in range(NB_CORES):
        bi, oi = divmod(c, OI)
        out[bi * BS:(bi + 1) * BS, oi * OS:(oi + 1) * OS] = res.results[c]["o"].T
    return out
